# revision 2
# baseline (speedup 1.0000x reference)
"""Toeplitz bias kernel for trn2 (8 NeuronCores).

bias[h, j, i] = p[h, 2047 + j - i],  p = exp(w_ - offset),  L = 2048,
S = 2L-1 = 4095.  With q = reverse(p):  bias[h, j, i] = q[h, (L-1-j) + i].

Host precomputes q (tiny: 16 heads x 16KB) and packs 64 SEED rows per
head: row k holds qpad[c - k] for c in [0, 4096) (qpad = q zero-padded
outside [0, S)).  Device work is then pure data movement building
qb[t, c] = qpad[c - t] for all 128 rows, done in COLUMN CHUNKS from high
columns to low so the first output stores start ~5us in and the rest of
the staircase construction hides behind them:

  chunk [cs, ce) (high -> low: [2047,4096), [1023,2047), [127,1023)):
    - load seed rows 0..63, cols [cs-64, ce)            (plain DMA)
    - copy rows 64..127 <- rows 0..63 shifted +64 cols  (SBUF->SBUF DMA)
    - issue the output-block stores this chunk enables:
        block b reads cols [c0, c0+L), c0 = 2047 - 128b
        [2047,4096) -> b=0; [1023,2047) -> b=1..8; [127,1023) -> b=9..15
        out[h, j0+t, i] = qb[t, (2047-j0) + i]  (1MB contiguous store)

All APs are plain (partition dim first, positive strides): Tile's
dependency tracker mishandles anything fancier. Store phase is HBM-bound
(~91-96us for 32MB/core); chunking keeps HBM busy from ~5us.

Heads are sharded 2 per core across the 8 cores; each core writes its own
[2, L, L] output and the host concatenates.
"""

import numpy as np

H = 16
L = 2048
S = 2 * L - 1  # 4095
CP = 4096  # padded cols per head in SBUF
N_CORES = 8
HPC = H // N_CORES  # heads per core
NBLK = L // 128  # 16 row blocks per head
SEEDS = 64

_cached_nc = None


def _build_nc(inner_reps=1, bufs=1):
    import concourse.bacc as bacc
    import concourse.mybir as mybir
    import concourse.tile as tile

    nc = bacc.Bacc("TRN2", target_bir_lowering=False)
    f32 = mybir.dt.float32
    win = nc.dram_tensor("win", [SEEDS, HPC, CP], f32, kind="ExternalInput")
    out = nc.dram_tensor("out", [HPC, L, L], f32, kind="ExternalOutput")

    with tile.TileContext(nc) as tc:
        with tc.tile_pool(name="p", bufs=bufs) as pool:
            for _rep in range(inner_reps):
                qb = pool.tile([128, HPC, CP], f32, tag="qb")
                rings = (nc.sync, nc.scalar)
                sengs3 = (nc.gpsimd, nc.sync, nc.scalar)
                snum = 0
                for cs, ce, blks in (
                    (2047, CP, range(0, 1)),
                    (1023, 2047, range(1, 9)),
                    (127, 1023, range(9, NBLK)),
                ):
                    lo = cs - 64
                    for h in range(HPC):
                        ring = rings[h % 2]
                        ring.dma_start(qb[0:64, h, lo:ce], win[:, h, lo:ce])
                        ring.dma_start(
                            qb[64:128, h, cs:ce], qb[0:64, h, lo : ce - 64]
                        )
                    for b in blks:
                        j0 = 128 * b
                        c0 = L - 1 - j0
                        for h in range(HPC):
                            if b == 0:
                                eng = nc.gpsimd
                            else:
                                eng = sengs3[snum % 3]
                                snum += 1
                            eng.dma_start(
                                out[h, j0 : j0 + 128, :],
                                qb[:, h, c0 : c0 + L],
                            )
    nc.compile()
    return nc


def _get_nc():
    global _cached_nc
    if _cached_nc is None:
        _cached_nc = _build_nc()
    return _cached_nc


def _make_in_maps(w_, offset):
    w_ = np.asarray(w_, dtype=np.float32)
    offset = np.asarray(offset, dtype=np.float32)
    # q[h, k] = p[h, S-1-k],  p = exp(w - offset)
    p = np.exp(w_ - offset[:, None], dtype=np.float32)
    q = p[:, ::-1]
    # seeds[k, h, c] = qpad[h, c - k], qpad zero outside [0, S)
    PAD = SEEDS - 1
    qpad = np.zeros((H, PAD + CP), dtype=np.float32)
    qpad[:, PAD : PAD + S] = q
    seeds_arr = np.empty((H, SEEDS, CP), dtype=np.float32)
    for k in range(SEEDS):
        seeds_arr[:, k, :] = qpad[:, PAD - k : PAD - k + CP]
    in_maps = []
    for c in range(N_CORES):
        sl = slice(c * HPC, (c + 1) * HPC)
        # [HPC, SEEDS, CP] -> [SEEDS, HPC, CP]
        in_maps.append(
            {"win": np.ascontiguousarray(seeds_arr[sl].transpose(1, 0, 2))}
        )
    return in_maps


def run(w_, offset, trace=False, **trace_kw):
    import concourse.bass_utils as bu
    from concourse.bass_utils import run_bass_kernel_spmd

    if trace:
        # no fish bucket in this container; keep artifacts local
        bu.upload_artifacts = lambda tmpdir: "local://" + str(tmpdir)

    nc = _get_nc()
    in_maps = _make_in_maps(w_, offset)
    res = run_bass_kernel_spmd(
        nc, in_maps, list(range(N_CORES)), trace=trace, **trace_kw
    )
    parts = [np.asarray(r["out"]) for r in res.results]
    full = np.concatenate(parts, axis=0)  # [H, L, L]
    return full, res


def kernel(w_, offset, seq_len=None, **_ignored):
    full, _ = run(w_, offset, trace=False)
    return full


def bench(w_, offset, reps=4, inner_reps=1, n_cores=N_CORES, bufs=1):
    """Slope-based per-rep estimate: pipelines `reps` executions of a NEFF
    holding `inner_reps` kernel bodies. Use two inner_reps values and take
    (t2-t1)/(ir2-ir1) to cancel the large axon dispatch constant."""
    import time

    import jax
    from jax.sharding import Mesh, PartitionSpec
    from jax.experimental.shard_map import shard_map

    import concourse.mybir as mybir
    from concourse import bass2jax

    bass2jax.install_neuronx_cc_hook()
    nc = _build_nc(inner_reps=inner_reps, bufs=bufs)
    in_maps = _make_in_maps(w_, offset)[:n_cores]

    partition_name = nc.partition_id_tensor.name if nc.partition_id_tensor else None
    in_names, out_names, out_avals, zero_outs = [], [], [], []
    for alloc in nc.m.functions[0].allocations:
        if not isinstance(alloc, mybir.MemoryLocationSet):
            continue
        name = alloc.memorylocations[0].name
        if alloc.kind == "ExternalInput":
            if name != partition_name:
                in_names.append(name)
        elif alloc.kind == "ExternalOutput":
            shape = tuple(alloc.tensor_shape)
            dtype = mybir.dt.np(alloc.dtype)
            out_names.append(name)
            out_avals.append(jax.core.ShapedArray(shape, dtype))
            zero_outs.append(np.zeros(shape, dtype))
    n_params = len(in_names)
    n_outs = len(out_avals)
    in_names_all = in_names + out_names
    if partition_name is not None:
        in_names_all.append(partition_name)

    def _body(*args):
        operands = list(args)
        if partition_name is not None:
            operands.append(bass2jax.partition_id_tensor())
        outs = bass2jax._bass_exec_p.bind(
            *operands,
            out_avals=tuple(out_avals),
            in_names=tuple(in_names_all),
            out_names=tuple(out_names),
            lowering_input_output_aliases=(),
            sim_require_finite=False,
            sim_require_nnan=False,
            nc=nc,
        )
        return tuple(outs)

    devices = jax.devices()[:n_cores]
    mesh = Mesh(np.asarray(devices), ("core",))
    in_specs = (PartitionSpec("core"),) * (n_params + n_outs)
    out_specs = (PartitionSpec("core"),) * n_outs
    donate = tuple(range(n_params, n_params + n_outs))
    sharded = jax.jit(
        shard_map(
            _body, mesh=mesh, in_specs=in_specs, out_specs=out_specs, check_rep=False
        ),
        donate_argnums=donate,
        keep_unused=True,
    )

    per_core = [[np.asarray(m[name]) for name in in_names] for m in in_maps]
    concat_in = [
        np.concatenate([per_core[c][i] for c in range(n_cores)], axis=0)
        for i in range(n_params)
    ]
    sharding = jax.sharding.NamedSharding(mesh, PartitionSpec("core"))
    dev_in = [jax.device_put(a, sharding) for a in concat_in]
    zshapes = [(n_cores * z.shape[0], *z.shape[1:]) for z in zero_outs]

    def fresh_zeros():
        return [
            jax.device_put(np.zeros(s, z.dtype), sharding)
            for s, z in zip(zshapes, zero_outs)
        ]

    warm = sharded(*dev_in, *fresh_zeros())
    out_np = [np.asarray(o) for o in warm]
    del warm

    staged = [fresh_zeros() for _ in range(reps)]
    for zs in staged:
        jax.block_until_ready(zs)

    t0 = time.perf_counter()
    last = None
    for zs in staged:
        last = sharded(*dev_in, *zs)
    jax.block_until_ready(last)
    t1 = time.perf_counter()
    total_ns = (t1 - t0) * 1e9
    est = total_ns / reps

    full = np.concatenate(
        [out_np[0].reshape(n_cores, HPC, L, L)[c] for c in range(n_cores)],
        axis=0,
    )
    return est, full
